# revision 7
# baseline (speedup 1.0000x reference)
"""GNN attention block (nn_AttentionBlock) on 8 Trainium2 NeuronCores.

Aligned-CSR design (v5):
  - Host shards receivers across cores (6250/core), sorts each core's
    receivers by in-degree (desc), and packs them into 49 windows of 128.
    Window w holds receivers of rank [128w, 128w+128); slot (p, j) of
    window w is the j-th incoming edge of the window's p-th receiver.
    L_w = max in-degree within window w (shared across cores so the SPMD
    program is identical; degree sorting keeps sum(L_w) ~= E/128).
  - Q is projected per receiver in rank order (phase 1) and stays SBUF
    resident: slot (p, j) reads Q from partition p directly -- no routing
    matmuls, no one-hot matrices, no per-edge Q copies.
  - K-pass: per-slot K projection on the TensorEngine (senders' x rows
    staged by host, transposed); batched PSUM->SBUF copies on the Scalar
    engine; logits/softmax numerator on the VectorEngine with 2x packed
    bf16 ops ((d, h) column order keeps heads packed in the last dim).
  - V-pass: per-slot V projection; V is weighted by e directly from PSUM
    (DVE/GpSimd alternating groups) and scatter-accumulated per receiver
    with identity-lhsT matmuls (partition-aligned combine).
  - Epilogue: denominator reciprocal, output projection, per-window DMA.
  - Host un-permutes the rank-ordered output rows at the end.
"""

import numpy as np
import ml_dtypes

N = 50000
M = 800000
H = 8
DK = 32
DV = 32
DE = 256
INV_SQRT_DK = float(1.0 / np.sqrt(DK))

NCORES = 8
NPC = N // NCORES            # 6250 receiver nodes per core
WPC = 49                     # 49 windows of 128 ranks (6272 = 49*128)
RPC = WPC * 128              # padded rank count per core
QPAD = 6656                  # 13 * 512 padded rank count for the Q phase
QT = QPAD // 512             # 13

BF16 = ml_dtypes.bfloat16

_CACHE = {}

# column permutation: new col (d*8 + h) <- old col (h*32 + d)
_PERM = ((np.arange(256) % 8) * 32 + (np.arange(256) // 8)).astype(np.int64)


def _build(LW, has_bkv, has_bq, has_bff):
    from concourse import bacc, tile, mybir

    LW = list(LW)
    SUML = sum(LW)
    LMAX = max(LW)
    f32, bf16 = mybir.dt.float32, mybir.dt.bfloat16
    Copy = mybir.ActivationFunctionType.Copy
    Exp = mybir.ActivationFunctionType.Exp
    AOT = mybir.AluOpType

    nc = bacc.Bacc("TRN2", target_bir_lowering=False, debug=False,
                   num_devices=NCORES)

    xeT = nc.dram_tensor("xeT", [128, 2, SUML * 128], bf16, kind="ExternalInput")
    xqT = nc.dram_tensor("xqT", [QT, 128, 2, 512], bf16, kind="ExternalInput")
    maskT = nc.dram_tensor("maskT", [128, SUML], bf16, kind="ExternalInput")
    wkv = nc.dram_tensor("wkv", [128, 1024], bf16, kind="ExternalInput")
    wq = nc.dram_tensor("wq", [128, 512], bf16, kind="ExternalInput")
    wff = nc.dram_tensor("wff", [128, 512], bf16, kind="ExternalInput")
    ident = nc.dram_tensor("ident", [128, 128], bf16, kind="ExternalInput")
    bkv = nc.dram_tensor("bkv", [1, 512], bf16, kind="ExternalInput")
    bq = nc.dram_tensor("bq", [1, 256], bf16, kind="ExternalInput")
    bff = nc.dram_tensor("bff", [1, 256], bf16, kind="ExternalInput")
    ones = nc.dram_tensor("ones", [1, 128], bf16, kind="ExternalInput")
    out = nc.dram_tensor("out", [RPC, 256], f32, kind="ExternalOutput")

    with tile.TileContext(nc) as tc:
        with tc.tile_pool(name="const", bufs=1) as cp:
            wkv_t = cp.tile([128, 1024], bf16)
            wq_t = cp.tile([128, 512], bf16)
            wff_t = cp.tile([128, 512], bf16)
            id_t = cp.tile([128, 128], bf16)
            bkv_t = cp.tile([1, 512], bf16)
            bq_t = cp.tile([1, 256], bf16)
            bff_t = cp.tile([1, 256], bf16)
            ones_t = cp.tile([1, 128], bf16)
            for t, src in ((wkv_t, wkv), (wq_t, wq), (wff_t, wff),
                           (id_t, ident), (bkv_t, bkv), (bq_t, bq),
                           (bff_t, bff), (ones_t, ones)):
                nc.sync.dma_start(out=t[:], in_=src[:])
            # Q rows per rank, SBUF-resident: rank w*128+p at [p, w, :]
            q_own = cp.tile([128, WPC + 3, 256], bf16)

            # ---- Phase 1: Q projection in rank order ----
            with tc.tile_pool(name="qsb", bufs=3) as qp, \
                 tc.tile_pool(name="qps", bufs=2, space="PSUM") as qpp:
                for t in range(QT):
                    xq_t = qp.tile([128, 2, 512], bf16)
                    nc.sync.dma_start(out=xq_t[:], in_=xqT[t])
                    for gg in range(2):
                        ps = qpp.tile([128, 2, 256], f32)
                        for g2 in range(2):
                            g = gg * 2 + g2
                            st = True
                            if has_bq:
                                nc.tensor.matmul(ps[:, g2, :], lhsT=ones_t[:],
                                                 rhs=bq_t[:], start=True, stop=False)
                                st = False
                            nc.tensor.matmul(ps[:, g2, :],
                                             lhsT=xq_t[:, 0, g * 128:(g + 1) * 128],
                                             rhs=wq_t[:, 0:256], start=st, stop=False)
                            nc.tensor.matmul(ps[:, g2, :],
                                             lhsT=xq_t[:, 1, g * 128:(g + 1) * 128],
                                             rhs=wq_t[:, 256:512], start=False, stop=True)
                        nc.scalar.activation(
                            q_own[:, 4 * t + 2 * gg: 4 * t + 2 * gg + 2, :],
                            ps[:], Copy)

            # ---- Phase 2: windows ----
            with tc.tile_pool(name="xep", bufs=2) as xep, \
                 tc.tile_pool(name="ksp", bufs=2) as ksp, \
                 tc.tile_pool(name="prp", bufs=2) as prp, \
                 tc.tile_pool(name="esp", bufs=2) as esp, \
                 tc.tile_pool(name="smp", bufs=2) as smp, \
                 tc.tile_pool(name="kvp", bufs=2, space="PSUM") as kvp, \
                 tc.tile_pool(name="accp", bufs=1, space="PSUM") as accp, \
                 tc.tile_pool(name="ffp", bufs=1, space="PSUM") as ffp:
                OFF = 0
                for w in range(WPC):
                    L = LW[w]
                    if L == 0:
                        continue
                    G = (L + 3) // 4
                    xe_t = xep.tile([128, 2, LMAX * 128], bf16)
                    nc.sync.dma_start(
                        out=xe_t[:, :, 0:L * 128],
                        in_=xeT[:, :, OFF * 128:(OFF + L) * 128])
                    m_t = smp.tile([128, LMAX], bf16)
                    nc.sync.dma_start(out=m_t[:, 0:L], in_=maskT[:, OFF:OFF + L])

                    k_sb = ksp.tile([128, LMAX, 256], bf16)
                    prod = prp.tile([128, LMAX, 256], bf16)
                    r16 = prp.tile([128, LMAX, 16, 8], bf16)
                    r8 = prp.tile([128, LMAX, 8, 8], bf16)
                    r4 = prp.tile([128, LMAX, 4, 8], bf16)
                    r2 = prp.tile([128, LMAX, 2, 8], bf16)
                    att = prp.tile([128, LMAX, 8], bf16)
                    e_m = smp.tile([128, LMAX, 8], bf16)
                    E_t = esp.tile([128, LMAX, 264], bf16)

                    # K-pass
                    for g in range(G):
                        gl = min(4, L - 4 * g)
                        kp = kvp.tile([128, 4, 256], f32, tag="kv")
                        for jj in range(gl):
                            j = 4 * g + jj
                            st = True
                            if has_bkv:
                                nc.tensor.matmul(kp[:, jj, :], lhsT=ones_t[:],
                                                 rhs=bkv_t[:, 0:256],
                                                 start=True, stop=False)
                                st = False
                            nc.tensor.matmul(kp[:, jj, :],
                                             lhsT=xe_t[:, 0, j * 128:(j + 1) * 128],
                                             rhs=wkv_t[:, 0:256], start=st, stop=False)
                            nc.tensor.matmul(kp[:, jj, :],
                                             lhsT=xe_t[:, 1, j * 128:(j + 1) * 128],
                                             rhs=wkv_t[:, 512:768], start=False, stop=True)
                        nc.scalar.activation(k_sb[:, 4 * g:4 * g + gl, :],
                                             kp[:, 0:gl, :], Copy)

                    # logits: prod = K * Q[p] ; tree-reduce over d (heads packed)
                    p4 = prod[:].rearrange("p l (d h) -> p l d h", h=8)
                    k4 = k_sb[:].rearrange("p l (d h) -> p l d h", h=8)
                    qv = q_own[:, w, :].rearrange("p (o d h) -> p o d h", o=1, h=8)
                    nc.vector.tensor_tensor(
                        out=p4[:, 0:L], in0=k4[:, 0:L],
                        in1=qv.to_broadcast([128, L, 32, 8]), op=AOT.mult)
                    nc.vector.tensor_tensor(out=r16[:, 0:L], in0=p4[:, 0:L, 0:16],
                                            in1=p4[:, 0:L, 16:32], op=AOT.add)
                    nc.vector.tensor_tensor(out=r8[:, 0:L], in0=r16[:, 0:L, 0:8],
                                            in1=r16[:, 0:L, 8:16], op=AOT.add)
                    nc.vector.tensor_tensor(out=r4[:, 0:L], in0=r8[:, 0:L, 0:4],
                                            in1=r8[:, 0:L, 4:8], op=AOT.add)
                    nc.vector.tensor_tensor(out=r2[:, 0:L], in0=r4[:, 0:L, 0:2],
                                            in1=r4[:, 0:L, 2:4], op=AOT.add)
                    nc.vector.tensor_tensor(out=att[:, 0:L, :].unsqueeze(2),
                                            in0=r2[:, 0:L, 0:1], in1=r2[:, 0:L, 1:2],
                                            op=AOT.add)
                    # e = exp(att/sqrt(dk)) * mask
                    e_sb = smp.tile([128, LMAX, 8], bf16)
                    nc.scalar.activation(e_sb[:, 0:L, :], att[:, 0:L, :], Exp,
                                         scale=INV_SQRT_DK)
                    nc.vector.tensor_tensor(
                        out=e_m[:, 0:L, :], in0=e_sb[:, 0:L, :],
                        in1=m_t[:, 0:L].unsqueeze(-1).to_broadcast([128, L, 8]),
                        op=AOT.mult)
                    nc.vector.tensor_scalar(out=E_t[:, 0:L, 256:264],
                                            in0=e_m[:, 0:L, :], scalar1=1.0,
                                            scalar2=None, op0=AOT.mult)

                    # V-pass + weighted combine
                    acc = accp.tile([128, 264], f32)
                    E4 = E_t[:, :, 0:256].rearrange("p l (d h) -> p l d h", h=8)
                    em4 = e_m[:].unsqueeze(2)
                    for g in range(G):
                        gl = min(4, L - 4 * g)
                        vp = kvp.tile([128, 4, 256], f32, tag="kv")
                        for jj in range(gl):
                            j = 4 * g + jj
                            st = True
                            if has_bkv:
                                nc.tensor.matmul(vp[:, jj, :], lhsT=ones_t[:],
                                                 rhs=bkv_t[:, 256:512],
                                                 start=True, stop=False)
                                st = False
                            nc.tensor.matmul(vp[:, jj, :],
                                             lhsT=xe_t[:, 0, j * 128:(j + 1) * 128],
                                             rhs=wkv_t[:, 256:512], start=st, stop=False)
                            nc.tensor.matmul(vp[:, jj, :],
                                             lhsT=xe_t[:, 1, j * 128:(j + 1) * 128],
                                             rhs=wkv_t[:, 768:1024], start=False, stop=True)
                        vp4 = vp[:].rearrange("p l (d h) -> p l d h", h=8)
                        eng = nc.vector if (g % 2 == 0) else nc.gpsimd
                        eng.tensor_tensor(
                            out=E4[:, 4 * g:4 * g + gl],
                            in0=vp4[:, 0:gl],
                            in1=em4[:, 4 * g:4 * g + gl].to_broadcast(
                                [128, gl, 32, 8]),
                            op=AOT.mult)
                        for jj in range(gl):
                            j = 4 * g + jj
                            nc.tensor.matmul(acc[:], lhsT=id_t[:], rhs=E_t[:, j, :],
                                             start=(j == 0), stop=(j == L - 1))

                    # epilogue
                    dsafe = smp.tile([128, 8], f32)
                    nc.vector.tensor_scalar(out=dsafe[:], in0=acc[:, 256:264],
                                            scalar1=1e-30, scalar2=None, op0=AOT.max)
                    rec = smp.tile([128, 8], f32)
                    nc.vector.reciprocal(rec[:], dsafe[:])
                    outpre = esp.tile([128, 256], bf16)
                    nc.vector.tensor_tensor(
                        out=outpre[:].rearrange("p (d h) -> p d h", h=8),
                        in0=acc[:, 0:256].rearrange("p (d h) -> p d h", h=8),
                        in1=rec[:].unsqueeze(1).to_broadcast([128, 32, 8]),
                        op=AOT.mult)
                    lhsT_ff = esp.tile([128, 2, 128], bf16)
                    for k in range(2):
                        psT = ffp.tile([128, 128], bf16, tag="ep")
                        nc.tensor.transpose(psT[:], outpre[:, k * 128:(k + 1) * 128],
                                            id_t[:])
                        nc.scalar.activation(lhsT_ff[:, k, :], psT[:], Copy)
                    ffps = ffp.tile([128, 256], f32, tag="ep")
                    st = True
                    if has_bff:
                        nc.tensor.matmul(ffps[:], lhsT=ones_t[:], rhs=bff_t[:],
                                         start=True, stop=False)
                        st = False
                    nc.tensor.matmul(ffps[:], lhsT=lhsT_ff[:, 0, :], rhs=wff_t[:, 0:256],
                                     start=st, stop=False)
                    nc.tensor.matmul(ffps[:], lhsT=lhsT_ff[:, 1, :], rhs=wff_t[:, 256:512],
                                     start=False, stop=True)
                    out_sb = esp.tile([128, 256], f32)
                    nc.scalar.activation(out_sb[:], ffps[:], Copy)
                    nc.sync.dma_start(out=out[w * 128:(w + 1) * 128, :], in_=out_sb[:])
                    OFF += L

    nc.compile()
    return nc


def _preprocess(x, edge_index, W_qkv, b_qkv, W_ff, b_ff):
    senders = np.asarray(edge_index[0], dtype=np.int64)
    receivers = np.asarray(edge_index[1], dtype=np.int64)
    x = np.asarray(x, dtype=np.float32)

    Wq = W_qkv[:DK * H]
    Wk = W_qkv[DK * H:2 * DK * H]
    Wv = W_qkv[2 * DK * H:]
    w_kv_p = np.concatenate([Wk[_PERM], Wv[_PERM]], axis=0)       # [512, 256]
    wkv_in = np.ascontiguousarray(
        w_kv_p.T.reshape(2, 128, 512).transpose(1, 0, 2).reshape(128, 1024)
    ).astype(BF16)
    wq_in = np.ascontiguousarray(
        Wq[_PERM].T.reshape(2, 128, 256).transpose(1, 0, 2).reshape(128, 512)
    ).astype(BF16)
    wff_in = np.ascontiguousarray(
        W_ff.T[_PERM].reshape(2, 128, 256).transpose(1, 0, 2).reshape(128, 512)
    ).astype(BF16)

    b_q = np.asarray(b_qkv[:DK * H], np.float32)[_PERM]
    b_kvv = np.concatenate([
        np.asarray(b_qkv[DK * H:2 * DK * H], np.float32)[_PERM],
        np.asarray(b_qkv[2 * DK * H:], np.float32)[_PERM]])
    b_f = np.asarray(b_ff, np.float32)
    has_bq = bool(np.any(b_q != 0))
    has_bkv = bool(np.any(b_kvv != 0))
    has_bff = bool(np.any(b_f != 0))

    consts = {
        "wkv": wkv_in, "wq": wq_in, "wff": wff_in,
        "ident": np.eye(128, dtype=np.float32).astype(BF16),
        "bkv": b_kvv[None, :].astype(BF16),
        "bq": b_q[None, :].astype(BF16),
        "bff": b_f[None, :].astype(BF16),
        "ones": np.ones((1, 128), BF16),
    }

    x_bf = x.astype(BF16)
    xpad = np.zeros((N + 1, DE), BF16)      # last row: pad (zeros)
    xpad[:N] = x_bf

    core = receivers // NPC
    local = receivers - core * NPC

    orders = []
    degs_sorted = []
    Ls_per_core = np.zeros((NCORES, WPC), np.int64)
    for c in range(NCORES):
        deg = np.bincount(local[core == c], minlength=RPC)  # pads have deg 0
        order = np.argsort(-deg, kind="stable")             # rank -> local id
        ds = deg[order]
        orders.append(order)
        degs_sorted.append(ds)
        Ls_per_core[c] = ds.reshape(WPC, 128).max(axis=1)
    LW = tuple(int(v) for v in Ls_per_core.max(axis=0))
    SUML = sum(LW)
    OFFS = np.zeros(WPC, np.int64)
    np.cumsum(LW[:-1], out=OFFS[1:])

    in_maps = []
    full_perm = np.empty((NCORES, RPC), np.int64)
    for c in range(NCORES):
        order = orders[c]
        ds = degs_sorted[c]
        full_perm[c] = order
        rankmap = np.empty(RPC, np.int64)
        rankmap[order] = np.arange(RPC)

        sel = core == c
        e_loc = local[sel]
        e_snd = senders[sel]
        e_rank = rankmap[e_loc]
        eo = np.argsort(e_rank, kind="stable")
        er = e_rank[eo]
        es = e_snd[eo]
        counts = np.bincount(er, minlength=RPC)
        starts = np.zeros(RPC + 1, np.int64)
        np.cumsum(counts, out=starts[1:])
        jpos = np.arange(er.shape[0], dtype=np.int64) - starts[er]
        wn = er >> 7
        p = er & 127
        colg = (OFFS[wn] + jpos) * 128 + p

        snd_slot = np.full(SUML * 128, N, np.int64)
        snd_slot[colg] = es
        xe = xpad[snd_slot]                                  # [SUML*128, 256]
        xeT_in = np.ascontiguousarray(
            xe.reshape(SUML, 128, 2, 128).transpose(3, 2, 0, 1).reshape(
                128, 2, SUML * 128))

        mask = np.zeros((128, SUML), np.float32)
        dsw = ds.reshape(WPC, 128)
        for w in range(WPC):
            L = LW[w]
            if L == 0:
                continue
            mask[:, OFFS[w]:OFFS[w] + L] = (
                np.arange(L)[None, :] < dsw[w][:, None])
        mask_in = mask.astype(BF16)

        xq = np.zeros((QPAD, DE), BF16)
        xloc = np.zeros((RPC, DE), BF16)
        real = order < NPC
        xloc[real] = x_bf[c * NPC + order[real]]
        xq[:RPC] = xloc
        xqT_in = np.ascontiguousarray(
            xq.reshape(QT, 512, 2, 128).transpose(0, 3, 2, 1))

        m = {"xeT": xeT_in, "xqT": xqT_in, "maskT": mask_in}
        m.update(consts)
        in_maps.append(m)

    meta = (LW, full_perm)
    return meta, (has_bkv, has_bq, has_bff), in_maps


def _build_from_meta(meta, *bias_flags):
    LW, _ = meta
    return _build(LW, *bias_flags)


def _run(nc, in_maps, trace=False):
    from concourse.bass_utils import run_bass_kernel_spmd
    return run_bass_kernel_spmd(nc, in_maps, core_ids=list(range(NCORES)),
                                trace=trace)


def _postprocess(meta, res, b_ff):
    LW, full_perm = meta
    b_f = np.asarray(b_ff, np.float32)
    full = np.empty((N, DE), np.float32)
    for c in range(NCORES):
        o = np.array(res.results[c]["out"])           # [RPC, 256] rank order
        for w in range(WPC):
            if LW[w] == 0:
                o[w * 128:(w + 1) * 128] = b_f[None, :]
        order = full_perm[c]
        real = order < NPC
        full[c * NPC + order[real]] = o[real]
    return full


def kernel(x, edge_index, W_qkv, b_qkv, W_ff, b_ff):
    meta, bias_flags, in_maps = _preprocess(x, edge_index, W_qkv, b_qkv,
                                            W_ff, b_ff)
    key = (meta[0],) + bias_flags
    if key not in _CACHE:
        _CACHE[key] = _build(meta[0], *bias_flags)
    nc = _CACHE[key]
    res = _run(nc, in_maps)
    return _postprocess(meta, res, b_ff)


# revision 9
# speedup vs baseline: 1.3964x; 1.3964x over previous
"""GNN attention block (nn_AttentionBlock) on 8 Trainium2 NeuronCores.

Aligned-CSR design (v5):
  - Host shards receivers across cores (6250/core), sorts each core's
    receivers by in-degree (desc), and packs them into 49 windows of 128.
    Window w holds receivers of rank [128w, 128w+128); slot (p, j) of
    window w is the j-th incoming edge of the window's p-th receiver.
    L_w = max in-degree within window w (shared across cores so the SPMD
    program is identical; degree sorting keeps sum(L_w) ~= E/128).
  - Q is projected per receiver in rank order (phase 1) and stays SBUF
    resident: slot (p, j) reads Q from partition p directly -- no routing
    matmuls, no one-hot matrices, no per-edge Q copies.
  - K-pass: per-slot K projection on the TensorEngine (senders' x rows
    staged by host, transposed); batched PSUM->SBUF copies on the Scalar
    engine; logits/softmax numerator on the VectorEngine with 2x packed
    bf16 ops ((d, h) column order keeps heads packed in the last dim).
  - V-pass: per-slot V projection; V is weighted by e directly from PSUM
    (DVE/GpSimd alternating groups) and scatter-accumulated per receiver
    with identity-lhsT matmuls (partition-aligned combine).
  - Epilogue: denominator reciprocal, output projection, per-window DMA.
  - Host un-permutes the rank-ordered output rows at the end.
"""

import numpy as np
import ml_dtypes

N = 50000
M = 800000
H = 8
DK = 32
DV = 32
DE = 256
INV_SQRT_DK = float(1.0 / np.sqrt(DK))

NCORES = 8
NPC = N // NCORES            # 6250 receiver nodes per core
WPC = 49                     # 49 windows of 128 ranks (6272 = 49*128)
RPC = WPC * 128              # padded rank count per core
QPAD = 6656                  # 13 * 512 padded rank count for the Q phase
QT = QPAD // 512             # 13

BF16 = ml_dtypes.bfloat16

_CACHE = {}

# column permutation: new col (d*8 + h) <- old col (h*32 + d)
_PERM = ((np.arange(256) % 8) * 32 + (np.arange(256) // 8)).astype(np.int64)


def _build(LW, has_bkv, has_bq, has_bff):
    from concourse import bacc, tile, mybir

    LW = list(LW)
    SUML = sum(LW)
    LMAX = max(LW)
    f32, bf16 = mybir.dt.float32, mybir.dt.bfloat16
    Copy = mybir.ActivationFunctionType.Copy
    Exp = mybir.ActivationFunctionType.Exp
    AOT = mybir.AluOpType

    nc = bacc.Bacc("TRN2", target_bir_lowering=False, debug=False,
                   num_devices=NCORES)

    xeT = nc.dram_tensor("xeT", [128, 2, SUML * 128], bf16, kind="ExternalInput")
    xqT = nc.dram_tensor("xqT", [QT, 128, 2, 512], bf16, kind="ExternalInput")
    maskT = nc.dram_tensor("maskT", [128, SUML], bf16, kind="ExternalInput")
    wkv = nc.dram_tensor("wkv", [128, 1024], bf16, kind="ExternalInput")
    wq = nc.dram_tensor("wq", [128, 512], bf16, kind="ExternalInput")
    wff = nc.dram_tensor("wff", [128, 512], bf16, kind="ExternalInput")
    ident = nc.dram_tensor("ident", [128, 128], bf16, kind="ExternalInput")
    bkv = nc.dram_tensor("bkv", [1, 512], bf16, kind="ExternalInput")
    bq = nc.dram_tensor("bq", [1, 256], bf16, kind="ExternalInput")
    bff = nc.dram_tensor("bff", [1, 256], bf16, kind="ExternalInput")
    ones = nc.dram_tensor("ones", [1, 128], bf16, kind="ExternalInput")
    out = nc.dram_tensor("out", [RPC, 256], f32, kind="ExternalOutput")

    with tile.TileContext(nc) as tc:
        with tc.tile_pool(name="const", bufs=1) as cp:
            wkv_t = cp.tile([128, 1024], bf16)
            wq_t = cp.tile([128, 512], bf16)
            wff_t = cp.tile([128, 512], bf16)
            id_t = cp.tile([128, 128], bf16)
            bkv_t = cp.tile([1, 512], bf16)
            bq_t = cp.tile([1, 256], bf16)
            bff_t = cp.tile([1, 256], bf16)
            ones_t = cp.tile([1, 128], bf16)
            for t, src in ((wkv_t, wkv), (wq_t, wq), (wff_t, wff),
                           (id_t, ident), (bkv_t, bkv), (bq_t, bq),
                           (bff_t, bff), (ones_t, ones)):
                nc.sync.dma_start(out=t[:], in_=src[:])
            # Q rows per rank, SBUF-resident: rank w*128+p at [p, w, :]
            q_own = cp.tile([128, WPC + 3, 256], bf16)

            # ---- Phase 1: Q projection in rank order ----
            with tc.tile_pool(name="qsb", bufs=3) as qp, \
                 tc.tile_pool(name="qps", bufs=2, space="PSUM") as qpp:
                for t in range(QT):
                    xq_t = qp.tile([128, 2, 512], bf16)
                    nc.sync.dma_start(out=xq_t[:], in_=xqT[t])
                    for gg in range(2):
                        ps = qpp.tile([128, 2, 256], f32)
                        for g2 in range(2):
                            g = gg * 2 + g2
                            st = True
                            if has_bq:
                                nc.tensor.matmul(ps[:, g2, :], lhsT=ones_t[:],
                                                 rhs=bq_t[:], start=True, stop=False)
                                st = False
                            nc.tensor.matmul(ps[:, g2, :],
                                             lhsT=xq_t[:, 0, g * 128:(g + 1) * 128],
                                             rhs=wq_t[:, 0:256], start=st, stop=False)
                            nc.tensor.matmul(ps[:, g2, :],
                                             lhsT=xq_t[:, 1, g * 128:(g + 1) * 128],
                                             rhs=wq_t[:, 256:512], start=False, stop=True)
                        nc.scalar.activation(
                            q_own[:, 4 * t + 2 * gg: 4 * t + 2 * gg + 2, :],
                            ps[:], Copy)

            # ---- Phase 2: 3-stage software-pipelined windows ----
            # stage A(w): DMA + K matmuls/copies + logits + exp  (PE never
            # waits on DVE); stage B(w): V matmuls + e-weighting + combine
            # (e ready one slot earlier); stage C(w): epilogue + FF + out.
            wins = [w for w in range(WPC) if LW[w] > 0]
            OFFS = {}
            o = 0
            for w in range(WPC):
                OFFS[w] = o
                o += LW[w]
            state = {}

            def stage_a(w):
                L = LW[w]
                OFF = OFFS[w]
                G = (L + 3) // 4
                xe_t = xep.tile([128, 2, LMAX * 128], bf16)
                nc.sync.dma_start(
                    out=xe_t[:, :, 0:L * 128],
                    in_=xeT[:, :, OFF * 128:(OFF + L) * 128])
                m_t = smp.tile([128, LMAX], bf16)
                nc.sync.dma_start(out=m_t[:, 0:L], in_=maskT[:, OFF:OFF + L])

                k_sb = ksp.tile([128, LMAX, 256], bf16)
                prod = prp.tile([128, LMAX, 256], bf16)
                r16 = prp.tile([128, LMAX, 16, 8], bf16)
                r8 = prp.tile([128, LMAX, 8, 8], bf16)
                r4 = prp.tile([128, LMAX, 4, 8], bf16)
                r2 = prp.tile([128, LMAX, 2, 8], bf16)
                att = prp.tile([128, LMAX, 8], bf16)
                e_m = smp.tile([128, LMAX, 8], bf16)
                E_t = esp.tile([128, LMAX, 264], bf16)

                for g in range(G):
                    gl = min(4, L - 4 * g)
                    kp = kvp.tile([128, 4, 256], f32, tag="kv")
                    for jj in range(gl):
                        j = 4 * g + jj
                        st = True
                        if has_bkv:
                            nc.tensor.matmul(kp[:, jj, :], lhsT=ones_t[:],
                                             rhs=bkv_t[:, 0:256],
                                             start=True, stop=False)
                            st = False
                        nc.tensor.matmul(kp[:, jj, :],
                                         lhsT=xe_t[:, 0, j * 128:(j + 1) * 128],
                                         rhs=wkv_t[:, 0:256], start=st, stop=False)
                        nc.tensor.matmul(kp[:, jj, :],
                                         lhsT=xe_t[:, 1, j * 128:(j + 1) * 128],
                                         rhs=wkv_t[:, 512:768], start=False, stop=True)
                    nc.scalar.activation(k_sb[:, 4 * g:4 * g + gl, :],
                                         kp[:, 0:gl, :], Copy)

                p4 = prod[:].rearrange("p l (d h) -> p l d h", h=8)
                k4 = k_sb[:].rearrange("p l (d h) -> p l d h", h=8)
                qv = q_own[:, w, :].rearrange("p (o d h) -> p o d h", o=1, h=8)
                nc.vector.tensor_tensor(
                    out=p4[:, 0:L], in0=k4[:, 0:L],
                    in1=qv.to_broadcast([128, L, 32, 8]), op=AOT.mult)
                nc.vector.tensor_tensor(out=r16[:, 0:L], in0=p4[:, 0:L, 0:16],
                                        in1=p4[:, 0:L, 16:32], op=AOT.add)
                nc.vector.tensor_tensor(out=r8[:, 0:L], in0=r16[:, 0:L, 0:8],
                                        in1=r16[:, 0:L, 8:16], op=AOT.add)
                nc.vector.tensor_tensor(out=r4[:, 0:L], in0=r8[:, 0:L, 0:4],
                                        in1=r8[:, 0:L, 4:8], op=AOT.add)
                nc.vector.tensor_tensor(out=r2[:, 0:L], in0=r4[:, 0:L, 0:2],
                                        in1=r4[:, 0:L, 2:4], op=AOT.add)
                nc.vector.tensor_tensor(out=att[:, 0:L, :].unsqueeze(2),
                                        in0=r2[:, 0:L, 0:1], in1=r2[:, 0:L, 1:2],
                                        op=AOT.add)
                e_sb = smp.tile([128, LMAX, 8], bf16)
                nc.scalar.activation(e_sb[:, 0:L, :], att[:, 0:L, :], Exp,
                                     scale=INV_SQRT_DK)
                nc.vector.tensor_tensor(
                    out=e_m[:, 0:L, :], in0=e_sb[:, 0:L, :],
                    in1=m_t[:, 0:L].unsqueeze(-1).to_broadcast([128, L, 8]),
                    op=AOT.mult)
                nc.vector.tensor_scalar(out=E_t[:, 0:L, 256:264],
                                        in0=e_m[:, 0:L, :], scalar1=1.0,
                                        scalar2=None, op0=AOT.mult)
                state[w] = {"xe": xe_t, "em": e_m, "E": E_t}

            def _vmm(w, g):
                L = LW[w]
                gl = min(4, L - 4 * g)
                xe_t = state[w]["xe"]
                vp = kvp.tile([128, 4, 256], f32, tag="kv")
                for jj in range(gl):
                    j = 4 * g + jj
                    st = True
                    if has_bkv:
                        nc.tensor.matmul(vp[:, jj, :], lhsT=ones_t[:],
                                         rhs=bkv_t[:, 256:512],
                                         start=True, stop=False)
                        st = False
                    nc.tensor.matmul(vp[:, jj, :],
                                     lhsT=xe_t[:, 0, j * 128:(j + 1) * 128],
                                     rhs=wkv_t[:, 256:512], start=st, stop=False)
                    nc.tensor.matmul(vp[:, jj, :],
                                     lhsT=xe_t[:, 1, j * 128:(j + 1) * 128],
                                     rhs=wkv_t[:, 768:1024], start=False, stop=True)
                return vp

            def stage_b(w):
                L = LW[w]
                G = (L + 3) // 4
                st_w = state[w]
                E_t, e_m = st_w["E"], st_w["em"]
                acc = accp.tile([128, 264], f32, tag="acc")
                E4 = E_t[:, :, 0:256].rearrange("p l (d h) -> p l d h", h=8)
                em4 = e_m[:].unsqueeze(2)
                vps = [None] * G
                vps[0] = _vmm(w, 0)
                if G > 1:
                    vps[1] = _vmm(w, 1)
                for g in range(G):
                    gl = min(4, L - 4 * g)
                    vp4 = vps[g][:].rearrange("p l (d h) -> p l d h", h=8)
                    eng = nc.vector if (g % 2 == 0) else nc.gpsimd
                    eng.tensor_tensor(
                        out=E4[:, 4 * g:4 * g + gl],
                        in0=vp4[:, 0:gl],
                        in1=em4[:, 4 * g:4 * g + gl].to_broadcast(
                            [128, gl, 32, 8]),
                        op=AOT.mult)
                    vps[g] = None
                    if g + 2 < G:
                        vps[g + 2] = _vmm(w, g + 2)
                    for jj in range(gl):
                        j = 4 * g + jj
                        nc.tensor.matmul(acc[:], lhsT=id_t[:], rhs=E_t[:, j, :],
                                         start=(j == 0), stop=(j == L - 1))
                st_w["acc"] = acc

            def stage_c(w):
                acc = state[w]["acc"]
                dsafe = smp.tile([128, 8], f32)
                nc.vector.tensor_scalar(out=dsafe[:], in0=acc[:, 256:264],
                                        scalar1=1e-30, scalar2=None, op0=AOT.max)
                rec = smp.tile([128, 8], f32)
                nc.vector.reciprocal(rec[:], dsafe[:])
                outpre = esp.tile([128, 256], bf16)
                nc.vector.tensor_tensor(
                    out=outpre[:].rearrange("p (d h) -> p d h", h=8),
                    in0=acc[:, 0:256].rearrange("p (d h) -> p d h", h=8),
                    in1=rec[:].unsqueeze(1).to_broadcast([128, 32, 8]),
                    op=AOT.mult)
                lhsT_ff = esp.tile([128, 2, 128], bf16)
                for k in range(2):
                    psT = ffp.tile([128, 128], bf16, tag="ep")
                    nc.tensor.transpose(psT[:], outpre[:, k * 128:(k + 1) * 128],
                                        id_t[:])
                    nc.scalar.activation(lhsT_ff[:, k, :], psT[:], Copy)
                ffps = ffp.tile([128, 256], f32, tag="ep")
                st = True
                if has_bff:
                    nc.tensor.matmul(ffps[:], lhsT=ones_t[:], rhs=bff_t[:],
                                     start=True, stop=False)
                    st = False
                nc.tensor.matmul(ffps[:], lhsT=lhsT_ff[:, 0, :], rhs=wff_t[:, 0:256],
                                 start=st, stop=False)
                nc.tensor.matmul(ffps[:], lhsT=lhsT_ff[:, 1, :], rhs=wff_t[:, 256:512],
                                 start=False, stop=True)
                out_sb = esp.tile([128, 256], f32)
                nc.scalar.activation(out_sb[:], ffps[:], Copy)
                nc.sync.dma_start(out=out[w * 128:(w + 1) * 128, :], in_=out_sb[:])
                del state[w]

            with tc.tile_pool(name="xep", bufs=2) as xep, \
                 tc.tile_pool(name="ksp", bufs=2) as ksp, \
                 tc.tile_pool(name="prp", bufs=2) as prp, \
                 tc.tile_pool(name="esp", bufs=2) as esp, \
                 tc.tile_pool(name="smp", bufs=3) as smp, \
                 tc.tile_pool(name="kvp", bufs=2, space="PSUM") as kvp, \
                 tc.tile_pool(name="accp", bufs=2, space="PSUM") as accp, \
                 tc.tile_pool(name="ffp", bufs=1, space="PSUM") as ffp:
                nw = len(wins)
                for i in range(nw + 2):
                    if i < nw:
                        stage_a(wins[i])
                    if 1 <= i < nw + 1:
                        stage_b(wins[i - 1])
                    if 2 <= i:
                        stage_c(wins[i - 2])

    nc.compile()
    return nc


def _preprocess(x, edge_index, W_qkv, b_qkv, W_ff, b_ff):
    senders = np.asarray(edge_index[0], dtype=np.int64)
    receivers = np.asarray(edge_index[1], dtype=np.int64)
    x = np.asarray(x, dtype=np.float32)

    Wq = W_qkv[:DK * H]
    Wk = W_qkv[DK * H:2 * DK * H]
    Wv = W_qkv[2 * DK * H:]
    w_kv_p = np.concatenate([Wk[_PERM], Wv[_PERM]], axis=0)       # [512, 256]
    wkv_in = np.ascontiguousarray(
        w_kv_p.T.reshape(2, 128, 512).transpose(1, 0, 2).reshape(128, 1024)
    ).astype(BF16)
    wq_in = np.ascontiguousarray(
        Wq[_PERM].T.reshape(2, 128, 256).transpose(1, 0, 2).reshape(128, 512)
    ).astype(BF16)
    wff_in = np.ascontiguousarray(
        W_ff.T[_PERM].reshape(2, 128, 256).transpose(1, 0, 2).reshape(128, 512)
    ).astype(BF16)

    b_q = np.asarray(b_qkv[:DK * H], np.float32)[_PERM]
    b_kvv = np.concatenate([
        np.asarray(b_qkv[DK * H:2 * DK * H], np.float32)[_PERM],
        np.asarray(b_qkv[2 * DK * H:], np.float32)[_PERM]])
    b_f = np.asarray(b_ff, np.float32)
    has_bq = bool(np.any(b_q != 0))
    has_bkv = bool(np.any(b_kvv != 0))
    has_bff = bool(np.any(b_f != 0))

    consts = {
        "wkv": wkv_in, "wq": wq_in, "wff": wff_in,
        "ident": np.eye(128, dtype=np.float32).astype(BF16),
        "bkv": b_kvv[None, :].astype(BF16),
        "bq": b_q[None, :].astype(BF16),
        "bff": b_f[None, :].astype(BF16),
        "ones": np.ones((1, 128), BF16),
    }

    x_bf = x.astype(BF16)
    xpad = np.zeros((N + 1, DE), BF16)      # last row: pad (zeros)
    xpad[:N] = x_bf

    core = receivers // NPC
    local = receivers - core * NPC

    orders = []
    degs_sorted = []
    Ls_per_core = np.zeros((NCORES, WPC), np.int64)
    for c in range(NCORES):
        deg = np.bincount(local[core == c], minlength=RPC)  # pads have deg 0
        order = np.argsort(-deg, kind="stable")             # rank -> local id
        ds = deg[order]
        orders.append(order)
        degs_sorted.append(ds)
        Ls_per_core[c] = ds.reshape(WPC, 128).max(axis=1)
    LW = tuple(int(v) for v in Ls_per_core.max(axis=0))
    SUML = sum(LW)
    OFFS = np.zeros(WPC, np.int64)
    np.cumsum(LW[:-1], out=OFFS[1:])

    in_maps = []
    full_perm = np.empty((NCORES, RPC), np.int64)
    for c in range(NCORES):
        order = orders[c]
        ds = degs_sorted[c]
        full_perm[c] = order
        rankmap = np.empty(RPC, np.int64)
        rankmap[order] = np.arange(RPC)

        sel = core == c
        e_loc = local[sel]
        e_snd = senders[sel]
        e_rank = rankmap[e_loc]
        eo = np.argsort(e_rank, kind="stable")
        er = e_rank[eo]
        es = e_snd[eo]
        counts = np.bincount(er, minlength=RPC)
        starts = np.zeros(RPC + 1, np.int64)
        np.cumsum(counts, out=starts[1:])
        jpos = np.arange(er.shape[0], dtype=np.int64) - starts[er]
        wn = er >> 7
        p = er & 127
        colg = (OFFS[wn] + jpos) * 128 + p

        snd_slot = np.full(SUML * 128, N, np.int64)
        snd_slot[colg] = es
        xe = xpad[snd_slot]                                  # [SUML*128, 256]
        xeT_in = np.ascontiguousarray(
            xe.reshape(SUML, 128, 2, 128).transpose(3, 2, 0, 1).reshape(
                128, 2, SUML * 128))

        mask = np.zeros((128, SUML), np.float32)
        dsw = ds.reshape(WPC, 128)
        for w in range(WPC):
            L = LW[w]
            if L == 0:
                continue
            mask[:, OFFS[w]:OFFS[w] + L] = (
                np.arange(L)[None, :] < dsw[w][:, None])
        mask_in = mask.astype(BF16)

        xq = np.zeros((QPAD, DE), BF16)
        xloc = np.zeros((RPC, DE), BF16)
        real = order < NPC
        xloc[real] = x_bf[c * NPC + order[real]]
        xq[:RPC] = xloc
        xqT_in = np.ascontiguousarray(
            xq.reshape(QT, 512, 2, 128).transpose(0, 3, 2, 1))

        m = {"xeT": xeT_in, "xqT": xqT_in, "maskT": mask_in}
        m.update(consts)
        in_maps.append(m)

    meta = (LW, full_perm)
    return meta, (has_bkv, has_bq, has_bff), in_maps


def _build_from_meta(meta, *bias_flags):
    LW, _ = meta
    return _build(LW, *bias_flags)


def _run(nc, in_maps, trace=False):
    from concourse.bass_utils import run_bass_kernel_spmd
    return run_bass_kernel_spmd(nc, in_maps, core_ids=list(range(NCORES)),
                                trace=trace)


def _postprocess(meta, res, b_ff):
    LW, full_perm = meta
    b_f = np.asarray(b_ff, np.float32)
    full = np.empty((N, DE), np.float32)
    for c in range(NCORES):
        o = np.array(res.results[c]["out"])           # [RPC, 256] rank order
        for w in range(WPC):
            if LW[w] == 0:
                o[w * 128:(w + 1) * 128] = b_f[None, :]
        order = full_perm[c]
        real = order < NPC
        full[c * NPC + order[real]] = o[real]
    return full


def kernel(x, edge_index, W_qkv, b_qkv, W_ff, b_ff):
    meta, bias_flags, in_maps = _preprocess(x, edge_index, W_qkv, b_qkv,
                                            W_ff, b_ff)
    key = (meta[0],) + bias_flags
    if key not in _CACHE:
        _CACHE[key] = _build(meta[0], *bias_flags)
    nc = _CACHE[key]
    res = _run(nc, in_maps)
    return _postprocess(meta, res, b_ff)
